# revision 77
# baseline (speedup 1.0000x reference)
"""Axial attention (B,H,W,C)=(8,128,128,256), 8 heads, for 8 trn2 NeuronCores.

Sharding: data-parallel over batch B=8 -> one batch element per core.
Per core, two passes over x[b]:
  phase A: attention along H (one sequence per column w), writes
           oh + bout0 + bout1 to a bf16 HBM scratch in (H,W,C) layout.
  phase B: attention along W (one sequence per row h), adds the scratch row
           and writes the final fp32 output row.

Groups of G=8 sequences; both passes run as ONE stitched software pipeline
(32 group-steps). QKV projections are residual-compensated fp8e4m3
DoubleRow matmuls: host splits x ~ x8 + rx and 16*W ~ w8 + wr, and the
three fp8 products x8*w8 + rx*w8 + x8*wr (0.5 PE cycles/row each via
DoubleRow's [128, 2-plane] contraction) recover better-than-bf16 accuracy
at half the bf16 PE cost. Weight scaling (x16) cancels through the exp
scale (e^-.5/256) and a 16.0 ones column fused into V' (softmax
denominator from the AV matmul); no max-subtraction (scores are O(1)).

Engine schedule (the core of the speedup over the naive emission): each
group-step emits [score-matmuls(g,k), exp(g,k-1), work-unit(k)] for
k=0..7, where the 8 work-units carry BOTH the previous group's attention
tail (AV, reciprocal+normalize, PE-transpose, out-proj, final add + DMA)
AND the next group's projections+exits. PSUM pools are engine-partitioned:
a 2x2-bank pool runs the score->exp stream at the Activation engine's
back-to-back rate; a 4x1-bank pool rotates everything else so PSUM-exit
drains (the true bottleneck: every PSUM result must leave through DVE or
Act) pipeline at drain rate. Exit balance: Act = 8 exps + Q^T + 2 V
copies; DVE = K^T + 2 V + normalize + reciprocal + otb (2x bf16 mode) +
final adds. Phase-B scratch reads are emitted only after every
overlapping phase-A scratch write is in program order (in-order queues
cannot wait on future instructions); output DMAs ride the idle GPSIMD
queue so input loads never queue behind them.

Simulated per-core time 425693 ns vs 489077 ns for the v1 kernel;
hardware rel-err 4.6e-3 (better than all-bf16: the fp8 residual pair
quantizes x/W more finely than bf16 does).

Toolchain notes: neuronxcc here accepts at most ONE sync-wait per
instruction (Tile multi-waits are legalized onto same-engine nops);
matmul outputs must be fp32 on TRN2; GPSIMD cannot touch PSUM; DMA
cannot read PSUM; TensorTensor may read at most one PSUM operand.
"""

import sys

sys.path.insert(0, "/opt/trn_rl_repo")

import numpy as np
import ml_dtypes

import concourse.bass as bass
import concourse.tile as tile
from concourse import mybir
from concourse.bass_utils import run_bass_kernel_spmd
from concourse.vector_clock import ScopedClock

F32 = mybir.dt.float32
BF16 = mybir.dt.bfloat16
F8 = mybir.dt.float8e4
AF = mybir.ActivationFunctionType
OP = mybir.AluOpType
DR = mybir.MatmulPerfMode.DoubleRow

H = 128
W = 128
C = 256
HEADS = 8
E = C // HEADS  # 32
T = 128  # sequence length for both axes
G = 8  # sequences processed per group
NG = W // G  # 16 groups per pass
GT = G * T  # 1024 tokens per group
EXP_SCALE = float(E) ** -0.5 / 256.0

# --- workaround: this toolchain's codegen accepts at most ONE sync-wait per
# instruction; redistribute extra waits onto preceding same-engine nops. ---

_MAXW = 1


def _patched_drain_and_barrier(self, tick_clock, wait_clock):
    probe = self.nc.sync.nop(nofuse=True)
    wait_clock.add_sem_waits(probe.ins, ScopedClock({None: tick_clock.global_clock}))
    conds = list(probe.ins.sync_info.on_wait)
    probe.ins.sync_info.on_wait = conds[:_MAXW]
    rest = conds[_MAXW:]
    while rest:
        n2 = self.nc.sync.nop(nofuse=True)
        if n2.ins.sync_info is None:
            n2.ins.sync_info = mybir.SyncInfo(on_wait=[], on_update=[])
        n2.ins.sync_info.on_wait = rest[:_MAXW]
        rest = rest[_MAXW:]
    self.nc.sync.drain()
    self.nc.all_engine_barrier()
    popped = self.nc._tile_sem_poison_stack.pop()
    assert popped is self._sem_poison
    self.nc.clear_and_free_semaphores(list(self.sems.allocated().values()))
    self.nc.all_engine_barrier()


tile.TileContext._drain_and_barrier = _patched_drain_and_barrier


_CTRL_OPS = ("InstNoOp", "InstDrain", "InstEventSemaphore", "InstCompareAndBranch")


def _split_waits(nc, limit=_MAXW, compute_limit=1):
    """Hoist extra sync-waits onto fresh nops directly before their owner."""
    n_split = 0
    for fn in nc.m.functions:
        for blk in fn.blocks:
            insts = blk.instructions
            out = []
            for inst in insts:
                si = inst.sync_info
                limit = (
                    _MAXW if type(inst).__name__ in _CTRL_OPS else compute_limit
                )
                if si is not None and len(si.on_wait) > limit:
                    waits = list(si.on_wait)
                    extra, keep = waits[:-limit], waits[-limit:]
                    k = 0
                    while extra:
                        nop = mybir.InstNoOp(
                            name=f"{inst.name}-wsplit{k}",
                            engine=inst.engine,
                            bass_nofuse=True,
                            sync_info=mybir.SyncInfo(
                                on_wait=extra[:limit], on_update=[]
                            ),
                        )
                        nc.register_instruction(nop, overwrite=True)
                        out.append(nop)
                        extra = extra[limit:]
                        k += 1
                        n_split += 1
                    si.on_wait = keep
                out.append(inst)
            blk.instructions = out
    return n_split


def _bcast_rows(handle_ap, rows):
    """AP that broadcasts a 1D dram tensor across `rows` partitions."""
    return bass.AP(
        tensor=handle_ap.tensor,
        offset=handle_ap.offset,
        ap=[[0, rows]] + [list(p) for p in handle_ap.ap],
    )


def _build():
    nc = bass.Bass("TRN2", target_bir_lowering=False, debug=False)

    # host-pre-quantized inputs: value + residual fp8 planes, [grp, chunk, c, tok]
    xta8 = nc.dram_tensor("xta8", [NG, 2, 128, GT], F8, kind="ExternalInput")
    xtar = nc.dram_tensor("xtar", [NG, 2, 128, GT], F8, kind="ExternalInput")
    xtc8 = nc.dram_tensor("xtc8", [NG, 2, 128, GT], F8, kind="ExternalInput")
    xtcr = nc.dram_tensor("xtcr", [NG, 2, 128, GT], F8, kind="ExternalInput")
    # weights: [c-in-chunk, chunk, 3C] fp8 (x16), + residual
    wqkv8 = {}
    wqkvr = {}
    for ax in (0, 1):
        wqkv8[ax] = nc.dram_tensor(f"wqkv8_{ax}", [128, 2, 3 * C], F8, kind="ExternalInput")
        wqkvr[ax] = nc.dram_tensor(f"wqkvr_{ax}", [128, 2, 3 * C], F8, kind="ExternalInput")
    wout0 = nc.dram_tensor("wout0", [C, C], BF16, kind="ExternalInput")
    wout1 = nc.dram_tensor("wout1", [C, C], BF16, kind="ExternalInput")
    bsum = nc.dram_tensor("bsum", [C], F32, kind="ExternalInput")
    out = nc.dram_tensor("out", [H, W, C], F32, kind="ExternalOutput")
    scratch = nc.dram_tensor("ohs", [H, W, C], BF16)

    out_ap = out.ap()
    sc_ap = scratch.ap()
    KC = 2  # contraction chunks

    with tile.TileContext(nc) as tc:
        with (
            tc.tile_pool(name="const", bufs=1) as const,
            tc.tile_pool(name="work", bufs=8) as work,
            tc.tile_pool(name="stp", bufs=4) as stp,
            tc.tile_pool(name="qkp", bufs=4) as qkp,
            tc.tile_pool(name="ebp", bufs=21) as ebp,
            tc.tile_pool(name="onp", bufs=6) as onp,
            tc.tile_pool(name="pssc", bufs=2, space="PSUM") as pssc,
            tc.tile_pool(name="psh", bufs=4, space="PSUM") as psh,
        ):
            # ---- constants ----
            ident = const.tile([128, 128], BF16, tag="ident")
            from concourse.masks import make_identity

            make_identity(nc, ident)
            bsum_sb = const.tile([128, C], F32, tag="bsum")
            nc.gpsimd.dma_start(out=bsum_sb, in_=_bcast_rows(bsum.ap(), 128))

            w8_sb = {}
            wr_sb = {}
            wout_sb = {}
            for ax in (0, 1):
                eng8 = nc.scalar if ax == 0 else nc.gpsimd
                engr = nc.sync if ax == 0 else nc.gpsimd
                t8 = const.tile([128, 2, 3 * C], F8, tag=f"w8_{ax}")
                eng8.dma_start(out=t8, in_=wqkv8[ax].ap())
                w8_sb[ax] = t8
                tr = const.tile([128, 2, 3 * C], F8, tag=f"wr_{ax}")
                engr.dma_start(out=tr, in_=wqkvr[ax].ap())
                wr_sb[ax] = tr
                wo2 = (wout0 if ax == 0 else wout1).ap().rearrange(
                    "(k p) n -> k p n", p=128
                )
                for k in range(KC):
                    t_o = const.tile([128, C], BF16, tag=f"wout{ax}{k}")
                    nc.gpsimd.dma_start(out=t_o, in_=wo2[k])
                    wout_sb[ax, k] = t_o

            # persistent V'-tiles (4 groups deep): ones columns = 16.0
            NVP = 4
            vp_bufs = []
            for i in range(NVP):
                vpb = const.tile([128, G, HEADS * (E + 1)], BF16, tag=f"vp{i}")
                vpb4 = vpb.rearrange("p s (h q) -> p s h q", q=E + 1)
                nc.vector.memset(vpb4[:, :, :, E : E + 1], 16.0)
                vp_bufs.append(vpb)

            def emit_dma(ax, grp):
                """Allocate group state + issue its input DMAs."""
                xt8_ap = (xta8 if ax == 0 else xtc8).ap()
                xtr_ap = (xtar if ax == 0 else xtcr).ap()
                j0 = grp * G
                stb8 = stp.tile([128, 2, GT], F8, tag="stb8")
                nc.sync.dma_start(
                    out=stb8, in_=xt8_ap[grp].rearrange("k p t -> p k t")
                )
                stbr = stp.tile([128, 2, GT], F8, tag="stbr")
                nc.sync.dma_start(
                    out=stbr, in_=xtr_ap[grp].rearrange("k p t -> p k t")
                )
                vp = vp_bufs[grp % NVP]
                return dict(
                    ax=ax,
                    j0=j0,
                    ohrow=None,
                    stb8=stb8,
                    stbr=stbr,
                    qkb={},
                    vp4=vp.rearrange("p s (h q) -> p s h q", q=E + 1),
                    ebq={},
                )

            def proj_qk(st, which, m, half, on_act):
                """One half-token-range projection chunk -> qkb[which, m]."""
                ax, stb8, stbr = st["ax"], st["stb8"], st["stbr"]
                w8, wr = w8_sb[ax], wr_sb[ax]
                if (which, m) not in st["qkb"]:
                    sb = qkp.tile([128, GT], BF16, tag=f"qk{which}{m}")
                    st["qkb"][which, m] = sb
                sb = st["qkb"][which, m]
                pp = psh.tile([128, 512], F32, tag="psh")
                co = which * C + m * 128
                hs = slice(half * 512, (half + 1) * 512)
                terms = ((stb8, w8), (stbr, w8), (stb8, wr))
                for ti, (xa, wa) in enumerate(terms):
                    nc.tensor.matmul(
                        pp,
                        wa[:, :, co : co + 128],
                        xa[:, :, hs],
                        start=(ti == 0),
                        stop=(ti == 2),
                        perf_mode=DR,
                    )
                if on_act:
                    nc.scalar.activation(out=sb[:, hs], in_=pp, func=AF.Copy)
                else:
                    nc.vector.tensor_copy(out=sb[:, hs], in_=pp)

            def proj_v(st, vt, on_act):
                """Two sequences of V' -> vp columns (ones columns persist)."""
                ax, stb8, stbr = st["ax"], st["stb8"], st["stbr"]
                w8, wr = w8_sb[ax], wr_sb[ax]
                vp4 = st["vp4"]
                vv = psh.tile([128, 2, C], F32, tag="psh")
                for si in range(2):
                    s = vt * 2 + si
                    ts = slice(s * T, (s + 1) * T)
                    terms = ((stb8, w8), (stbr, w8), (stb8, wr))
                    for ti, (xa, wa) in enumerate(terms):
                        nc.tensor.matmul(
                            vv[:, si, :],
                            xa[:, :, ts],
                            wa[:, :, 2 * C : 3 * C],
                            start=(ti == 0),
                            stop=(ti == 2),
                            perf_mode=DR,
                        )
                dst = vp4[:, vt * 2 : (vt + 1) * 2, :, 0:E]
                srcv = vv.rearrange("p s (h e) -> p s h e", e=E)
                if on_act:
                    nc.scalar.activation(out=dst, in_=srcv, func=AF.Copy)
                else:
                    nc.vector.tensor_copy(out=dst, in_=srcv)

            def emit_proj_all(st):
                for half in range(2):
                    for m in range(KC):
                        proj_qk(st, 0, m, half, True)
                        proj_qk(st, 1, m, half, False)
                for vt in range(4):
                    proj_v(st, vt, vt % 2 == 0)

            def emit_scores(st, k):
                hg, q = divmod(k, 4)
                off = q * E
                qkb = st["qkb"]
                scq = pssc.tile([128, GT], F32, tag="pssc")
                for s in range(G):
                    ts = slice(s * T, (s + 1) * T)
                    nc.tensor.matmul(
                        scq[:, ts],
                        qkb[1, hg][off : off + E, ts],
                        qkb[0, hg][off : off + E, ts],
                        start=True,
                        stop=True,
                        tile_position=(off, 0),
                    )
                st["scq", k] = scq  # noqa

            def emit_exp(st, k):
                hg, q = divmod(k, 4)
                eb = ebp.tile([128, GT], BF16, tag="eb")
                nc.scalar.activation(
                    out=eb, in_=st.pop(("scq", k)), func=AF.Exp, scale=EXP_SCALE
                )
                st["ebq"][hg, q] = eb

            def emit_av(st, pair, hg):
                ebq, vp4 = st["ebq"], st["vp4"]
                quad = pair // 2
                if hg == 0 and pair % 2 == 0:
                    onorm_t = onp.tile([128, 4, C], BF16, tag="onorm")
                    st["onorm", quad] = onorm_t
                opp = psh.tile([128, 2 * C], F32, tag="psh")
                for si in range(2):
                    s = pair * 2 + si
                    ts = slice(s * T, (s + 1) * T)
                    for q in range(4):
                        nc.tensor.matmul(
                            opp[
                                :,
                                si * C + q * (E + 1) : si * C + (q + 1) * (E + 1),
                            ],
                            ebq[hg, q][:, ts],
                            vp4[:, s, hg * 4 + q, :],
                            start=True,
                            stop=True,
                        )
                om = opp[:]
                den_ap = bass.AP(
                    tensor=om.tensor,
                    offset=om.offset + E,
                    ap=[list(om.ap[0]), [C, 2], [E + 1, 4]],
                )
                num_ap = bass.AP(
                    tensor=om.tensor,
                    offset=om.offset,
                    ap=[list(om.ap[0]), [C, 2], [E + 1, 4], [1, E]],
                )
                recip = work.tile([128, 2, 4], F32, tag="recip")
                nc.vector.reciprocal(out=recip, in_=den_ap)
                ro = recip[:]
                rb = bass.AP(
                    tensor=ro.tensor,
                    offset=ro.offset,
                    ap=[list(p) for p in ro.ap] + [[0, E]],
                )
                onm = st["onorm", quad][:]
                out_ap_n = bass.AP(
                    tensor=onm.tensor,
                    offset=onm.offset + (pair % 2) * 2 * C + hg * 128,
                    ap=[list(onm.ap[0]), [C, 2], [E, 4], [1, E]],
                )
                nc.vector.tensor_tensor(
                    out=out_ap_n, in0=num_ap, in1=rb, op=OP.mult
                )

            def emit_transp(st, quad):
                onorm = st["onorm", quad]
                ot = psh.tile([128, GT], BF16, tag="psh")
                for si in range(4):
                    for k in range(KC):
                        nc.tensor.transpose(
                            ot[:, (si * 2 + k) * 128 : (si * 2 + k + 1) * 128],
                            onorm[:, si, k * 128 : (k + 1) * 128],
                            ident,
                        )
                otb = work.tile([128, GT], BF16, tag="otb")
                nc.vector.tensor_copy(out=otb, in_=ot)
                st["otb", quad] = otb

            def emit_outproj(st, pair):
                ax, j0, ohrow = st["ax"], st["j0"], st["ohrow"]
                quad = pair // 2
                otb = st["otb", quad]
                fps = psh.tile([128, 2, C], F32, tag="psh")
                for si in range(2):
                    sq = (pair % 2) * 2 + si
                    for k in range(KC):
                        nc.tensor.matmul(
                            fps[:, si, :],
                            otb[:, (sq * 2 + k) * 128 : (sq * 2 + k + 1) * 128],
                            wout_sb[ax, k],
                            start=(k == 0),
                            stop=(k == KC - 1),
                        )
                if ax == 0:
                    og = work.tile([128, 2, C], BF16, tag="oga")
                    bs = bsum_sb[:]
                    in1 = bass.AP(
                        tensor=bs.tensor,
                        offset=bs.offset,
                        ap=[list(bs.ap[0]), [0, 2], list(bs.ap[1])],
                    )
                else:
                    og = work.tile([128, 2, C], F32, tag="ogb")
                    in1 = ohrow[:, pair * 2 : pair * 2 + 2, :]
                nc.vector.tensor_tensor(out=og, in0=fps, in1=in1, op=OP.add)
                wq0 = j0 + pair * 2
                if ax == 0:
                    nc.gpsimd.dma_start(out=sc_ap[:, wq0 : wq0 + 2, :], in_=og)
                else:
                    nc.gpsimd.dma_start(
                        out=out_ap[wq0 : wq0 + 2].rearrange("h w c -> w h c"),
                        in_=og,
                    )

            def emit_ohrow(st):
                """Scratch-row fetch for a phase-B group; emitted only once
                every scratch write it overlaps is already in program order
                (the group is about to run its tail)."""
                if st["ax"] == 1 and st["ohrow"] is None:
                    j0 = st["j0"]
                    ohrow = work.tile([128, G, C], BF16, tag="ohrow")
                    nc.gpsimd.dma_start(
                        out=ohrow,
                        in_=sc_ap[j0 : j0 + G].rearrange("h w c -> w h c"),
                    )
                    st["ohrow"] = ohrow

            def build_units(tail_st, proj_st):
                """Per-k work units: previous group's tail + next group's
                projections, interleaved into the score/exp stream."""

                def u(*fns):
                    def run():
                        for f in fns:
                            f()
                    return run

                t = tail_st
                p = proj_st
                units = [[] for _ in range(8)]
                if t is not None:
                    units[0].append(lambda: emit_av(t, 0, 0))
                    units[0].append(lambda: emit_av(t, 0, 1))
                    units[1].append(lambda: emit_av(t, 1, 0))
                    units[1].append(lambda: emit_av(t, 1, 1))
                    units[2].append(lambda: emit_transp(t, 0))
                    units[3].append(lambda: emit_outproj(t, 0))
                    units[3].append(lambda: emit_outproj(t, 1))
                    units[4].append(lambda: emit_av(t, 2, 0))
                    units[4].append(lambda: emit_av(t, 2, 1))
                    units[5].append(lambda: emit_av(t, 3, 0))
                    units[5].append(lambda: emit_av(t, 3, 1))
                    units[6].append(lambda: emit_transp(t, 1))
                    units[7].append(lambda: emit_outproj(t, 2))
                    units[7].append(lambda: emit_outproj(t, 3))
                if p is not None:
                    units[0].append(lambda: proj_qk(p, 0, 0, 0, True))
                    units[0].append(lambda: proj_qk(p, 1, 0, 0, False))
                    units[1].append(lambda: proj_qk(p, 0, 0, 1, True))
                    units[1].append(lambda: proj_qk(p, 1, 0, 1, False))
                    units[2].append(lambda: proj_qk(p, 0, 1, 0, True))
                    units[2].append(lambda: proj_qk(p, 1, 1, 0, False))
                    units[4].append(lambda: proj_qk(p, 0, 1, 1, True))
                    units[4].append(lambda: proj_qk(p, 1, 1, 1, False))
                    units[5].append(lambda: proj_v(p, 0, True))
                    units[5].append(lambda: proj_v(p, 1, False))
                    units[6].append(lambda: proj_v(p, 2, True))
                    units[6].append(lambda: proj_v(p, 3, False))
                return [u(*fs) for fs in units]

            def full_schedule():
                st_cur = emit_dma(0, 0)
                # prologue: only the m0 projection pair (gates the first
                # scores); the rest streams through the first loop's units
                for half in range(2):
                    proj_qk(st_cur, 0, 0, half, True)
                    proj_qk(st_cur, 1, 0, half, False)
                st_tail = None
                for gi in range(2 * NG):
                    last = gi + 1 >= 2 * NG
                    if not last:
                        ax2, grp2 = divmod(gi + 1, NG)
                        st_next = emit_dma(ax2, grp2)
                    else:
                        st_next = None
                    if st_tail is not None:
                        emit_ohrow(st_tail)
                    units = build_units(st_tail, st_next)
                    extra = {}
                    if gi == 0:
                        sc0 = st_cur
                        extra = {
                            0: lambda: (
                                proj_qk(sc0, 0, 1, 0, True),
                                proj_qk(sc0, 1, 1, 0, False),
                            ),
                            1: lambda: (
                                proj_qk(sc0, 0, 1, 1, True),
                                proj_qk(sc0, 1, 1, 1, False),
                            ),
                            2: lambda: (
                                proj_v(sc0, 0, True),
                                proj_v(sc0, 1, False),
                            ),
                            3: lambda: (
                                proj_v(sc0, 2, True),
                                proj_v(sc0, 3, False),
                            ),
                        }
                    if last:
                        # pull the final group's hg0 AV work into its own loop
                        sc = st_cur
                        extra = {
                            4: lambda: emit_av(sc, 0, 0),
                            5: lambda: emit_av(sc, 1, 0),
                            6: lambda: emit_av(sc, 2, 0),
                            7: lambda: emit_av(sc, 3, 0),
                        }
                    for k in range(8):
                        emit_scores(st_cur, k)
                        units[k]()
                        if k > 0:
                            emit_exp(st_cur, k - 1)
                        if k in extra:
                            extra[k]()
                    emit_exp(st_cur, 7)
                    st_tail, st_cur = st_cur, st_next
                emit_ohrow(st_tail)
                t = st_tail
                for pair in range(4):
                    emit_av(t, pair, 1)
                    if pair % 2 == 1:
                        emit_transp(t, pair // 2)
                emit_outproj(t, 0)
                emit_outproj(t, 1)
                emit_outproj(t, 2)
                emit_outproj(t, 3)

            full_schedule()

    _split_waits(nc)
    return nc


_NC = None


def _get_nc():
    global _NC
    if _NC is None:
        _NC = _build()
    return _NC


def make_in_maps(x, Wq0, Wkv0, Wout0, bout0, Wq1, Wkv1, Wout1, bout1):
    bf = ml_dtypes.bfloat16
    f8 = ml_dtypes.float8_e4m3
    x = np.asarray(x, dtype=np.float32)

    def wsplit(Wq, Wkv):
        wfull = np.concatenate([Wq, Wkv], axis=1).astype(np.float32) * 16.0
        w8 = wfull.astype(f8)
        wr = (wfull - w8.astype(np.float32)).astype(f8)
        # [c, 3C] -> [c-in-chunk, chunk, 3C]
        def lay(w):
            return np.ascontiguousarray(
                w.reshape(2, 128, 3 * C).transpose(1, 0, 2)
            )
        return lay(w8), lay(wr)

    w8_0, wr_0 = wsplit(Wq0, Wkv0)
    w8_1, wr_1 = wsplit(Wq1, Wkv1)
    shared = {
        "wqkv8_0": w8_0,
        "wqkvr_0": wr_0,
        "wqkv8_1": w8_1,
        "wqkvr_1": wr_1,
        "wout0": np.asarray(Wout0, dtype=bf),
        "wout1": np.asarray(Wout1, dtype=bf),
        "bsum": np.asarray(bout0 + bout1, dtype=np.float32),
    }

    maps = []
    for b in range(x.shape[0]):
        e = x[b]  # (H, W, C) fp32
        x8 = e.astype(f8)
        xr = (e - x8.astype(np.float32)).astype(f8)

        def lay_a(z):
            # xta[g, i, p, s*T+t] = z[t, G*g+s, 128i+p]
            return np.ascontiguousarray(
                z.transpose(1, 2, 0)  # (W, C, H)
                .reshape(NG, G, 2, 128, H)
                .transpose(0, 2, 3, 1, 4)
            ).reshape(NG, 2, 128, GT)

        def lay_c(z):
            # xtc[g, i, p, s*T+w] = z[G*g+s, w, 128i+p]
            return np.ascontiguousarray(
                z.reshape(NG, G, W, 2, 128).transpose(0, 3, 4, 1, 2)
            ).reshape(NG, 2, 128, GT)

        maps.append(
            {
                "xta8": lay_a(x8),
                "xtar": lay_a(xr),
                "xtc8": lay_c(x8),
                "xtcr": lay_c(xr),
                **shared,
            }
        )
    return maps


def kernel(x, Wq0, Wkv0, Wout0, bout0, Wq1, Wkv1, Wout1, bout1):
    nc = _get_nc()
    in_maps = make_in_maps(
        np.asarray(x),
        np.asarray(Wq0),
        np.asarray(Wkv0),
        np.asarray(Wout0),
        np.asarray(bout0, dtype=np.float32),
        np.asarray(Wq1),
        np.asarray(Wkv1),
        np.asarray(Wout1),
        np.asarray(bout1, dtype=np.float32),
    )
    res = run_bass_kernel_spmd(nc, in_maps, core_ids=list(range(8)))
    return np.stack([r["out"] for r in res.results]).astype(np.float32)
